# revision 65
# baseline (speedup 1.0000x reference)
"""Trainium2 Bass kernel for GQA causal sliding-window self-attention.

Sharding: 8 cores = 2 (batch) x 4 (KV-head groups). Each core handles one
batch element and one KV head with its 3 GQA query heads. The output
projection is computed per-group against the matching Wproj column slice;
the 4 partial outputs per batch are summed on the host.

Everything on-chip runs in feature-major ("transposed") layout so that all
matmul contractions have their contraction dim on SBUF partitions and all
DRAM traffic is contiguous. fp32r matmuls (full-rate) with fp32 PSUM
accumulation. Sliding-window/causal masking is applied by accumulating a
host-precomputed -1e9 additive mask tile into the scores PSUM via an
identity matmul (exp then underflows to exactly 0).
"""

import os
import sys
import numpy as np

sys.path.insert(0, "/opt/trn_rl_repo")

from contextlib import ExitStack

from concourse import mybir, bacc, tile
from concourse.bass_utils import run_bass_kernel_spmd

f32 = mybir.dt.float32
f32r = mybir.dt.float32r
AF = mybir.ActivationFunctionType

B, T, C = 2, 2048, 1536
H, KV, D = 12, 4, 128
REP = H // KV          # 3 query heads per kv head
QD = REP * D           # 384
VE_GATE_CH = 12
N_CORES = 8
TC = 512               # t-chunk width (matmul moving free dim)
NTC = T // TC          # 4
NCC = C // 128         # 12 contraction chunks
NST = T // 128         # 16 s-tiles

_EPS = float(np.finfo(np.float32).eps)
# all scale constants folded into the q-side rsqrt:
#   rq = (1.2*1.2/sqrt(D)) * rsqrt(mean(q^2)+eps),  rk = rsqrt(mean(k^2)+eps)
_LNCQ = float(np.log(1.2 * 1.2 / np.sqrt(D)))
_MASKVAL = -1.0e9

_CACHE = {}


def _setup_act_tables():
    """Reorder activation-table sets so ln+exp share one set (avoids ~33
    table reloads).  Patches both the bacc-side set picker and the walrus
    --act-root-json (they must agree on set indices)."""
    try:
        import json
        import tempfile
        import concourse.hw_specs as hw_specs
        import concourse.bacc as bacc_mod
        from neuronxcc.driver.Job import Job
        from neuronxcc.driver.jobs.support.FindActInfo import findActInfoFile

        src = findActInfoFile(Job.getPackageDir(), "gen3")
        if not src or not os.path.exists(src):
            return
        src_dir = os.path.dirname(src)
        dst = os.path.join(tempfile.gettempdir(), "bass_act_pwp_lnexp")
        os.makedirs(dst, exist_ok=True)
        for f in os.listdir(src_dir):
            tgt = os.path.join(dst, f)
            if not os.path.exists(tgt):
                try:
                    os.symlink(os.path.join(src_dir, f), tgt)
                except OSError:
                    pass
        d = json.load(open(src))
        sets = d["act_func_sets"]
        idx = [i for i, s in enumerate(sets)
               if s["name"] == "natural_log_exp_and_others"]
        if not idx:
            return
        sets.insert(0, sets.pop(idx[0]))
        jp = os.path.join(dst, "act_info.json")
        if os.path.lexists(jp):
            os.remove(jp)
        json.dump(d, open(jp, "w"))
        os.environ["BASS_ACT_ROOT_JSON_PATH"] = jp

        orig = hw_specs.get_activation_tables

        def reordered(arch):
            t = orig(arch)
            key = "natural_log_exp_and_others"
            if key in t:
                out = {key: t[key]}
                out.update((k, v) for k, v in t.items() if k != key)
                return out
            return t

        hw_specs.get_activation_tables = reordered
        bacc_mod.get_activation_tables = reordered
    except Exception:
        pass


_setup_act_tables()


def _partial_deltas(window, win_finite):
    """Tile-offset classes (delta = t0 - s0) that need an additive mask."""
    deltas = set()
    for dlt in range(-(TC - 128), 0 + 1, 128):        # causal partials
        deltas.add(dlt)
    if win_finite:
        dlt = window - (window % 128)                  # window partials
        while dlt + (TC - 1) > window:
            if dlt >= -(TC - 128):
                deltas.add(dlt)
            dlt -= 128
    return sorted(deltas)


def _build(window: int):
    win_finite = 0 <= window < T
    deltas = _partial_deltas(window, win_finite)
    wdeltas = [d for d in deltas if win_finite and d > window - (TC - 1)]
    wmin = min(wdeltas) if wdeltas else 0
    WIDE = TC + (TC - 128)                  # covers 4 deltas of 128
    NM = 2 if wdeltas else 1

    nc = bacc.Bacc("TRN2", target_bir_lowering=False, debug=False,
                   num_devices=N_CORES)

    xT = nc.dram_tensor("xT", [C, T], f32r, kind="ExternalInput")
    wqT = nc.dram_tensor("wqT", [C, QD], f32r, kind="ExternalInput")
    wkT = nc.dram_tensor("wkT", [C, D], f32r, kind="ExternalInput")
    wvT = nc.dram_tensor("wvT", [C, D], f32r, kind="ExternalInput")
    wpT = nc.dram_tensor("wpT", [QD, C], f32r, kind="ExternalInput")
    wg = nc.dram_tensor("wg", [VE_GATE_CH, 1], f32r, kind="ExternalInput")
    veT = nc.dram_tensor("veT", [D, T], f32r, kind="ExternalInput")
    cos2 = nc.dram_tensor("cos2", [128, T], f32r, kind="ExternalInput")
    sin2m = nc.dram_tensor("sin2m", [128, T], f32r, kind="ExternalInput")
    eye = nc.dram_tensor("eye", [128, 128], f32r, kind="ExternalInput")
    onesI = nc.dram_tensor("onesI", [128, 1], f32r, kind="ExternalInput")
    masksI = nc.dram_tensor("masksI", [NM * 128, WIDE], f32r, kind="ExternalInput")
    outT = nc.dram_tensor("outT", [C, T], f32, kind="ExternalOutput")

    with tile.TileContext(nc) as tc, ExitStack() as ctx:
        # ---- persistent SBUF pools ----
        pw = ctx.enter_context(tc.tile_pool(name="pw", bufs=1))
        pbig = ctx.enter_context(tc.tile_pool(name="pbig", bufs=1))
        prow = ctx.enter_context(tc.tile_pool(name="prow", bufs=6))
        pbc = ctx.enter_context(tc.tile_pool(name="pbc", bufs=4))

        # ---- PSUM pools (8 banks total, elastic shared tags) ----
        psAO = ctx.enter_context(tc.tile_pool(name="psAO", bufs=1, space="PSUM"))
        psR = ctx.enter_context(tc.tile_pool(name="psR", bufs=2, space="PSUM"))
        psSY = ctx.enter_context(tc.tile_pool(name="psSY", bufs=5, space="PSUM"))

        # small constants (needed from phase 1)
        wg_sb = pw.tile([VE_GATE_CH, 1], f32r, tag="wg")
        nc.sync.dma_start(wg_sb[:], wg.ap()[:])
        ones_sb = pw.tile([128, 1], f32r, tag="ones")
        nc.sync.dma_start(ones_sb[:], onesI.ap()[:])
        eye_sb = pw.tile([128, 128], f32r, tag="eye")
        nc.sync.dma_start(eye_sb[:], eye.ap()[:])
        masks_sb = pw.tile([128, NM, WIDE], f32r, tag="masks")
        eps_row = pw.tile([128, 1], f32, tag="epsr")
        nc.vector.memset(eps_row[:], _EPS)
        lncq_row = pw.tile([128, 1], f32, tag="lncq")
        nc.vector.memset(lncq_row[:], _LNCQ)

        # big persistent activations
        qT_sb = [pbig.tile([128, T], f32r, tag=f"qT{m}", name=f"qT{m}")
                 for m in range(REP)]
        kT_sb = pbig.tile([128, T], f32r, tag="kT")
        V_sb = pbig.tile([128, NST, D], f32r, tag="V")

        xT_re = xT.ap().rearrange("(cc p) t -> p cc t", p=128)

        # =========== phase 1: projections + gate + rope + rmsnorm ===========
        pending_pe = []   # deferred PE ops (sumsq matmuls, v transposes)

        def flush_pe(n=None):
            k = len(pending_pe) if n is None else min(n, len(pending_pe))
            for _ in range(k):
                pending_pe.pop(0)()

        with ExitStack() as ctx1:
            p1w = ctx1.enter_context(tc.tile_pool(name="p1w", bufs=1))
            pxt = ctx1.enter_context(tc.tile_pool(name="pxt", bufs=5))
            pcs = ctx1.enter_context(tc.tile_pool(name="pcs", bufs=2))
            ptmp = ctx1.enter_context(tc.tile_pool(name="ptmp", bufs=12))

            wk_sb = p1w.tile([128, NCC, D], f32r, tag="wk")
            wkT_re = wkT.ap().rearrange("(cc p) m -> p cc m", p=128)
            for g0 in range(0, NCC, 6):
                nc.sync.dma_start(wk_sb[:, g0:g0 + 6, :], wkT_re[:, g0:g0 + 6, :])
            wv_sb = p1w.tile([128, NCC, D], f32r, tag="wv")
            wvT_re = wvT.ap().rearrange("(cc p) m -> p cc m", p=128)

            HTC = TC // 2

            def load_xt(tci):
                eng = nc.sync
                halves = []
                for hh in range(2):
                    t0 = tci * TC + hh * HTC
                    xth = pxt.tile([128, NCC, HTC], f32r, tag="xt", name="xth")
                    for g0 in range(0, NCC, 4):
                        eng.dma_start(xth[:, g0:g0 + 4, :],
                                      xT_re[:, g0:g0 + 4, t0:t0 + HTC])
                    halves.append(xth)
                return halves

            xt_next = load_xt(0)
            for g0 in range(0, NCC, 6):
                nc.scalar.dma_start(wv_sb[:, g0:g0 + 6, :], wvT_re[:, g0:g0 + 6, :])
            wq_sb = p1w.tile([128, NCC, QD], f32r, tag="wq")
            wqT_re = wqT.ap().rearrange("(cc p) m -> p cc m", p=128)
            for g0 in range(0, NCC, 3):
                nc.sync.dma_start(wq_sb[:, g0:g0 + 3, :], wqT_re[:, g0:g0 + 3, :])

            for tci in range(NTC):
                t0 = tci * TC
                xt = xt_next
                cs = pcs.tile([128, TC], f32r, tag="cs")
                nc.sync.dma_start(cs[:], cos2.ap()[:, t0:t0 + TC])
                sn = pcs.tile([128, TC], f32r, tag="sn")
                nc.sync.dma_start(sn[:], sin2m.ap()[:, t0:t0 + TC])
                ve_t = pcs.tile([128, TC], f32r, tag="vet")
                nc.sync.dma_start(ve_t[:], veT.ap()[:, t0:t0 + TC])
                if tci + 1 < NTC:
                    xt_next = load_xt(tci + 1)

                # ve gate: sigmoid(x[:, :12] @ wg); the *3 is folded into veT
                zg = psR.tile([1, TC], f32, tag="row")
                nc.tensor.matmul(zg[0:1, 0:HTC], wg_sb[:],
                                 xt[0][0:VE_GATE_CH, 0, :],
                                 start=True, stop=False)
                nc.tensor.matmul(zg[0:1, HTC:TC], wg_sb[:],
                                 xt[1][0:VE_GATE_CH, 0, :],
                                 start=False, stop=True)
                ez = prow.tile([1, TC], f32, tag="g")
                nc.scalar.activation(ez[:], zg[:], AF.Exp, scale=-1.0)
                ez1 = prow.tile([1, TC], f32, tag="g")
                nc.vector.tensor_scalar_add(ez1[:], ez[:], 1.0)
                grow = prow.tile([1, TC], f32, tag="g")
                nc.vector.reciprocal(grow[:], ez1[:])
                gbc = pbc.tile([128, TC], f32, tag="bc")
                nc.gpsimd.partition_broadcast(gbc[:], grow[:])

                streams = [("k", 0)] + [("q", m) for m in range(REP)] + [("v", 0)]
                for kind, m in streams:
                    acc = psSY.tile([128, TC], f32, tag="sy", name="acc")
                    for hh in range(2):
                        for cc in range(NCC):
                            if kind == "q":
                                lhsT = wq_sb[:, cc, m * D:(m + 1) * D]
                            elif kind == "k":
                                lhsT = wk_sb[:, cc, :]
                            else:
                                lhsT = wv_sb[:, cc, :]
                            nc.tensor.matmul(
                                acc[:, hh * HTC:(hh + 1) * HTC], lhsT,
                                xt[hh][:, cc, :],
                                start=(cc == 0 and hh == 0),
                                stop=(cc == NCC - 1 and hh == 1))

                    if kind == "v":
                        # v += gate * ve; then transpose into natural [s, D]
                        vtmp = ptmp.tile([128, TC], f32, tag="t")
                        nc.vector.tensor_mul(vtmp[:], gbc[:], ve_t[:])
                        vfull = ptmp.tile([128, TC], f32r, tag="t")
                        nc.vector.tensor_add(vfull[:], vtmp[:], acc[:])

                        def vtrans(tci=tci, vfull=vfull):
                            for j in range(TC // 128):
                                st = tci * (TC // 128) + j
                                vtr = psSY.tile([128, 128], f32r, tag="sy",
                                                name="vtr")
                                nc.tensor.transpose(
                                    vtr[:], vfull[:, j * 128:(j + 1) * 128],
                                    eye_sb[:])
                                if j % 2 == 0:
                                    nc.scalar.copy(V_sb[:, st, :], vtr[:])
                                else:
                                    nc.vector.tensor_copy(V_sb[:, st, :], vtr[:])
                        pending_pe.append(vtrans)
                        continue

                    # q/k: evacuate PSUM early, then rmsnorm stats off SBUF
                    qraw = ptmp.tile([128, TC], f32r, tag="t")
                    nc.scalar.copy(qraw[:], acc[:])
                    sqr = ptmp.tile([128, TC], f32r, tag="t")
                    nc.scalar.activation(sqr[:], qraw[:], AF.Square)

                    def final(kind=kind, m=m, qraw=qraw, sqr=sqr, t0=t0,
                              cs=cs, sn=sn):
                        ss = psR.tile([1, TC], f32, tag="row", name="ss")
                        nc.tensor.matmul(ss[:], ones_sb[:], sqr[:],
                                         start=True, stop=True)
                        lnr = prow.tile([1, TC], f32, tag="r", name="lnr")
                        nc.scalar.activation(lnr[:], ss[:], AF.Ln,
                                             scale=1.0 / D, bias=eps_row[0:1, :])
                        rr = prow.tile([1, TC], f32, tag="r", name="rr")
                        if kind == "q":
                            nc.scalar.activation(rr[:], lnr[:], AF.Exp,
                                                 scale=-0.5,
                                                 bias=lncq_row[0:1, :])
                        else:
                            nc.scalar.activation(rr[:], lnr[:], AF.Exp,
                                                 scale=-0.5, bias=0.0)
                        rbc = pbc.tile([128, TC], f32, tag="bc", name="rbc")
                        nc.gpsimd.partition_broadcast(rbc[:], rr[:])

                        qn = ptmp.tile([128, TC], f32r, tag="t", name="qn")
                        nc.vector.tensor_mul(qn[:], rbc[:], qraw[:])
                        # rope: out = qn*[cos;cos] + swap(qn)*[sin;-sin]
                        qsw = ptmp.tile([128, TC], f32r, tag="t", name="qsw")
                        nc.sync.dma_start(qsw[0:64, :], qn[64:128, :])
                        nc.sync.dma_start(qsw[64:128, :], qn[0:64, :])
                        ta = ptmp.tile([128, TC], f32r, tag="t", name="ta")
                        nc.vector.tensor_mul(ta[:], qn[:], cs[:])
                        tb = ptmp.tile([128, TC], f32r, tag="t", name="tb")
                        nc.vector.tensor_mul(tb[:], qsw[:], sn[:])
                        dst = qT_sb[m] if kind == "q" else kT_sb
                        nc.vector.tensor_add(dst[:, t0:t0 + TC], ta[:], tb[:])
                    pending_pe.append(final)

                    # keep PE dense: flush one deferred op per stream
                    if len(pending_pe) > 1:
                        flush_pe(1)
                if tci == 1:
                    nc.scalar.dma_start(
                        masks_sb[:],
                        masksI.ap().rearrange("(nd p) t -> p nd t", p=128))
            flush_pe()

        # =========== phase 2+3 per t-chunk: attention + out-proj ===========
        pw2 = ctx.enter_context(tc.tile_pool(name="pw2", bufs=1))
        wp_sb = pw2.tile([128, REP, C], f32r, tag="wp")
        nc.scalar.dma_start(wp_sb[:], wpT.ap().rearrange("(qc p) c -> p qc c",
                                                         p=128))
        yT_sb = [pw2.tile([128, T], f32r, tag=f"yT{m}", name=f"yT{m}")
                 for m in range(REP)]
        pP = ctx.enter_context(tc.tile_pool(name="pP", bufs=6))
        pout = ctx.enter_context(tc.tile_pool(name="pout", bufs=3))

        for tci in range(NTC):
            t0 = tci * TC
            if win_finite:
                st_min = max(0, (t0 - window - 127) // 128 + 1)
            else:
                st_min = 0
            st_max = (t0 + TC - 1) // 128
            sts = list(range(st_min, st_max + 1))

            for h in range(REP):
                yU = psSY.tile([128, TC], f32, tag="sy", name="yU")
                den = psR.tile([1, TC], f32, tag="row", name="den")
                q_rhs = qT_sb[h][:, t0:t0 + TC]
                pends = []    # software-pipeline den/Y two s-tiles behind
                for idx, st in enumerate(sts):
                    s0 = st * 128
                    delta = t0 - s0
                    causal_p = delta <= 0
                    window_p = win_finite and delta > window - (TC - 1)
                    nmm = int(causal_p) + int(window_p)
                    sc = psSY.tile([128, TC], f32, tag="sy", name="sc")
                    nc.tensor.matmul(sc[:], kT_sb[:, s0:s0 + 128], q_rhs,
                                     start=True, stop=(nmm == 0))
                    if causal_p:    # masked cols [0, 128-delta)
                        c0, c1 = 0, min(TC, 128 - delta)
                        off = delta + (TC - 128)
                        nmm -= 1
                        nc.tensor.matmul(sc[:, c0:c1], eye_sb[:],
                                         masks_sb[:, 0, off + c0:off + c1],
                                         start=False, stop=(nmm == 0))
                    if window_p:    # masked cols suffix
                        c0 = min(TC - 128, (window - delta + 1) // 128 * 128)
                        c1 = TC
                        off = delta - wmin
                        nmm -= 1
                        nc.tensor.matmul(sc[:, c0:c1], eye_sb[:],
                                         masks_sb[:, 1, off + c0:off + c1],
                                         start=False, stop=(nmm == 0))
                    if len(pends) >= 2:
                        pends.pop(0)()
                    P = pP.tile([128, TC], f32r, tag="P", name="P")
                    nc.scalar.activation(P[:], sc[:], AF.Exp)

                    def mk(idx=idx, st=st, P=P):
                        first, last = idx == 0, idx == len(sts) - 1
                        def go():
                            nc.tensor.matmul(den[:], ones_sb[:], P[:],
                                             start=first, stop=last)
                            nc.tensor.matmul(yU[:], V_sb[:, st, :], P[:],
                                             start=first, stop=last)
                        return go
                    pends.append(mk())
                while pends:
                    pends.pop(0)()
                dinv = prow.tile([1, TC], f32, tag="r", name="dinv")
                nc.vector.reciprocal(dinv[:], den[:])
                dbc = pbc.tile([128, TC], f32, tag="bc", name="dbc")
                nc.gpsimd.partition_broadcast(dbc[:], dinv[:])
                nc.vector.tensor_mul(yT_sb[h][:, t0:t0 + TC], dbc[:], yU[:])

            # ---- out-proj for this t-chunk ----
            for cc in range(NCC):
                if tci == NTC - 1 and cc % 2 == 1:
                    o = psSY.tile([128, TC], f32, tag="sy", name="o")
                else:
                    o = psAO.tile([128, TC], f32, tag="ao", name="o")
                for m in range(REP):
                    nc.tensor.matmul(o[:], wp_sb[:, m, cc * 128:(cc + 1) * 128],
                                     yT_sb[m][:, t0:t0 + TC],
                                     start=(m == 0), stop=(m == REP - 1))
                ot = pout.tile([128, TC], f32, tag="ot", name="ot")
                if cc % 2 == 0:
                    nc.vector.tensor_copy(ot[:], o[:])
                else:
                    nc.scalar.copy(ot[:], o[:])
                nc.sync.dma_start(outT.ap()[cc * 128:(cc + 1) * 128,
                                            t0:t0 + TC], ot[:])

    nc.compile()
    nc._mask_cfg = {"wide": WIDE, "cmin": -(TC - 128), "wmin": wmin}
    return nc


def _prep_inputs(nc, window, x, ve, cos, sin, Wq, Wk, Wv, Wproj, Wg):
    """Build the 8 per-core input maps (host-side sharding + transposes)."""
    win_finite = 0 <= window < T
    cosT = np.ascontiguousarray(cos.reshape(T, D // 2).T)
    sinT = np.ascontiguousarray(sin.reshape(T, D // 2).T)
    cos2 = np.concatenate([cosT, cosT], axis=0)
    sin2m = np.concatenate([sinT, -sinT], axis=0)
    eye = np.eye(128, dtype=np.float32)
    ones = np.ones((128, 1), dtype=np.float32)

    ds = np.arange(128)[:, None]
    wcfg = nc._mask_cfg
    j = np.arange(wcfg["wide"])[None, :]
    mc = np.where(j + wcfg["cmin"] - ds >= 0, 0.0, _MASKVAL).astype(np.float32)
    rows = [mc]
    if win_finite:
        mw = np.where(j + wcfg["wmin"] - ds <= window, 0.0,
                      _MASKVAL).astype(np.float32)
        rows.append(mw)
    masks = np.concatenate(rows, axis=0)

    xTb = [np.ascontiguousarray(x[b].T) for b in range(B)]

    in_maps = []
    for core in range(N_CORES):
        b, g = divmod(core, KV)
        sl_q = slice(g * QD, (g + 1) * QD)
        sl_d = slice(g * D, (g + 1) * D)
        in_maps.append({
            "xT": xTb[b],
            "wqT": np.ascontiguousarray(Wq[sl_q].T),
            "wkT": np.ascontiguousarray(Wk[sl_d].T),
            "wvT": np.ascontiguousarray(Wv[sl_d].T),
            "wpT": np.ascontiguousarray(Wproj[:, sl_q].T),
            "wg": np.ascontiguousarray(Wg[g].reshape(VE_GATE_CH, 1)),
            "veT": np.ascontiguousarray(3.0 * ve[b, :, sl_d].T),
            "cos2": cos2, "sin2m": sin2m, "eye": eye, "onesI": ones,
            "masksI": masks,
        })
    return in_maps


def kernel(x, ve, cos, sin, Wq, Wk, Wv, Wproj, Wg, window, _trace=False):
    window = int(window)
    if window not in _CACHE:
        _CACHE[window] = _build(window)
    nc = _CACHE[window]

    in_maps = _prep_inputs(nc, window,
                           np.asarray(x, np.float32), np.asarray(ve, np.float32),
                           np.asarray(cos, np.float32), np.asarray(sin, np.float32),
                           np.asarray(Wq, np.float32), np.asarray(Wk, np.float32),
                           np.asarray(Wv, np.float32), np.asarray(Wproj, np.float32),
                           np.asarray(Wg, np.float32))

    res = run_bass_kernel_spmd(nc, in_maps, core_ids=list(range(N_CORES)),
                               trace=_trace)

    out = np.empty((B, T, C), dtype=np.float32)
    for b in range(B):
        acc = res.results[b * KV]["outT"].copy()
        for g in range(1, KV):
            acc += res.results[b * KV + g]["outT"]
        out[b] = acc.T
    if _trace:
        kernel._last_trace = res
    return out


# revision 71
# speedup vs baseline: 1.0333x; 1.0333x over previous
"""Trainium2 Bass kernel for GQA causal sliding-window self-attention.

Sharding: 8 cores = 2 (batch) x 4 (KV-head groups). Each core handles one
batch element and one KV head with its 3 GQA query heads. The output
projection is computed per-group against the matching Wproj column slice;
the 4 partial outputs per batch are summed on the host.

Everything on-chip runs in feature-major ("transposed") layout so that all
matmul contractions have their contraction dim on SBUF partitions and all
DRAM traffic is contiguous. fp32r matmuls (full-rate) with fp32 PSUM
accumulation. Sliding-window/causal masking is applied by accumulating a
host-precomputed -1e9 additive mask tile into the scores PSUM via an
identity matmul (exp then underflows to exactly 0).
"""

import os
import sys
import numpy as np

sys.path.insert(0, "/opt/trn_rl_repo")

from contextlib import ExitStack

from concourse import mybir, bacc, tile
from concourse.bass_utils import run_bass_kernel_spmd

f32 = mybir.dt.float32
f32r = mybir.dt.float32r
AF = mybir.ActivationFunctionType

B, T, C = 2, 2048, 1536
H, KV, D = 12, 4, 128
REP = H // KV          # 3 query heads per kv head
QD = REP * D           # 384
VE_GATE_CH = 12
N_CORES = 8
TC = 512               # t-chunk width (matmul moving free dim)
NTC = T // TC          # 4
NCC = C // 128         # 12 contraction chunks
NST = T // 128         # 16 s-tiles

_EPS = float(np.finfo(np.float32).eps)
# all scale constants folded into the q-side rsqrt:
#   rq = (1.2*1.2/sqrt(D)) * rsqrt(mean(q^2)+eps),  rk = rsqrt(mean(k^2)+eps)
_LNCQ = float(np.log(1.2 * 1.2 / np.sqrt(D)))
_MASKVAL = -1.0e9

_CACHE = {}


def _setup_act_tables():
    """Reorder activation-table sets so ln+exp share one set (avoids ~33
    table reloads).  Patches both the bacc-side set picker and the walrus
    --act-root-json (they must agree on set indices)."""
    try:
        import json
        import tempfile
        import concourse.hw_specs as hw_specs
        import concourse.bacc as bacc_mod
        from neuronxcc.driver.Job import Job
        from neuronxcc.driver.jobs.support.FindActInfo import findActInfoFile

        src = findActInfoFile(Job.getPackageDir(), "gen3")
        if not src or not os.path.exists(src):
            return
        src_dir = os.path.dirname(src)
        dst = os.path.join(tempfile.gettempdir(), "bass_act_pwp_lnexp")
        os.makedirs(dst, exist_ok=True)
        for f in os.listdir(src_dir):
            tgt = os.path.join(dst, f)
            if not os.path.exists(tgt):
                try:
                    os.symlink(os.path.join(src_dir, f), tgt)
                except OSError:
                    pass
        d = json.load(open(src))
        sets = d["act_func_sets"]
        idx = [i for i, s in enumerate(sets)
               if s["name"] == "natural_log_exp_and_others"]
        if not idx:
            return
        sets.insert(0, sets.pop(idx[0]))
        jp = os.path.join(dst, "act_info.json")
        if os.path.lexists(jp):
            os.remove(jp)
        json.dump(d, open(jp, "w"))
        os.environ["BASS_ACT_ROOT_JSON_PATH"] = jp

        orig = hw_specs.get_activation_tables

        def reordered(arch):
            t = orig(arch)
            key = "natural_log_exp_and_others"
            if key in t:
                out = {key: t[key]}
                out.update((k, v) for k, v in t.items() if k != key)
                return out
            return t

        hw_specs.get_activation_tables = reordered
        bacc_mod.get_activation_tables = reordered
    except Exception:
        pass


_setup_act_tables()


def _partial_deltas(window, win_finite):
    """Tile-offset classes (delta = t0 - s0) that need an additive mask."""
    deltas = set()
    for dlt in range(-(TC - 128), 0 + 1, 128):        # causal partials
        deltas.add(dlt)
    if win_finite:
        dlt = window - (window % 128)                  # window partials
        while dlt + (TC - 1) > window:
            if dlt >= -(TC - 128):
                deltas.add(dlt)
            dlt -= 128
    return sorted(deltas)


def _build(window: int):
    win_finite = 0 <= window < T
    deltas = _partial_deltas(window, win_finite)
    wdeltas = [d for d in deltas if win_finite and d > window - (TC - 1)]
    wmin = min(wdeltas) if wdeltas else 0
    WIDE = TC + (TC - 128)                  # covers 4 deltas of 128
    NM = 2 if wdeltas else 1

    nc = bacc.Bacc("TRN2", target_bir_lowering=False, debug=False,
                   num_devices=N_CORES)

    xT = nc.dram_tensor("xT", [C, T], f32r, kind="ExternalInput")
    wqT = nc.dram_tensor("wqT", [C, QD], f32r, kind="ExternalInput")
    wkT = nc.dram_tensor("wkT", [C, D], f32r, kind="ExternalInput")
    wvT = nc.dram_tensor("wvT", [C, D], f32r, kind="ExternalInput")
    wpT = nc.dram_tensor("wpT", [QD, C], f32r, kind="ExternalInput")
    wg = nc.dram_tensor("wg", [VE_GATE_CH, 1], f32r, kind="ExternalInput")
    veT = nc.dram_tensor("veT", [D, T], f32r, kind="ExternalInput")
    cos2 = nc.dram_tensor("cos2", [128, T], f32r, kind="ExternalInput")
    sin2m = nc.dram_tensor("sin2m", [128, T], f32r, kind="ExternalInput")
    eye = nc.dram_tensor("eye", [128, 128], f32r, kind="ExternalInput")
    onesI = nc.dram_tensor("onesI", [128, 1], f32r, kind="ExternalInput")
    masksI = nc.dram_tensor("masksI", [NM * 128, WIDE], f32r, kind="ExternalInput")
    outT = nc.dram_tensor("outT", [C, T], f32, kind="ExternalOutput")

    with tile.TileContext(nc) as tc, ExitStack() as ctx:
        # ---- persistent SBUF pools ----
        pw = ctx.enter_context(tc.tile_pool(name="pw", bufs=1))
        pbig = ctx.enter_context(tc.tile_pool(name="pbig", bufs=1))
        prow = ctx.enter_context(tc.tile_pool(name="prow", bufs=6))
        pbc = ctx.enter_context(tc.tile_pool(name="pbc", bufs=4))

        # ---- PSUM pools (8 banks total, elastic shared tags) ----
        psAO = ctx.enter_context(tc.tile_pool(name="psAO", bufs=1, space="PSUM"))
        psR = ctx.enter_context(tc.tile_pool(name="psR", bufs=2, space="PSUM"))
        psSY = ctx.enter_context(tc.tile_pool(name="psSY", bufs=5, space="PSUM"))

        # small constants (needed from phase 1)
        wg_sb = pw.tile([VE_GATE_CH, 1], f32r, tag="wg")
        nc.sync.dma_start(wg_sb[:], wg.ap()[:])
        ones_sb = pw.tile([128, 1], f32r, tag="ones")
        nc.sync.dma_start(ones_sb[:], onesI.ap()[:])
        eye_sb = pw.tile([128, 128], f32r, tag="eye")
        nc.sync.dma_start(eye_sb[:], eye.ap()[:])
        masks_sb = pw.tile([128, NM, WIDE], f32r, tag="masks")
        eps_row = pw.tile([128, 1], f32, tag="epsr")
        nc.vector.memset(eps_row[:], _EPS)
        lncq_row = pw.tile([128, 1], f32, tag="lncq")
        nc.vector.memset(lncq_row[:], _LNCQ)

        # big persistent activations
        qT_sb = [pbig.tile([128, T], f32r, tag=f"qT{m}", name=f"qT{m}")
                 for m in range(REP)]
        kT_sb = pbig.tile([128, T], f32r, tag="kT")
        V_sb = pbig.tile([128, NST, D], f32r, tag="V")

        xT_re = xT.ap().rearrange("(cc p) t -> p cc t", p=128)

        # =========== phase 1: projections + gate + rope + rmsnorm ===========
        pending_pe = []   # deferred PE ops (sumsq matmuls, v transposes)

        def flush_pe(n=None):
            k = len(pending_pe) if n is None else min(n, len(pending_pe))
            for _ in range(k):
                pending_pe.pop(0)()

        with ExitStack() as ctx1:
            p1w = ctx1.enter_context(tc.tile_pool(name="p1w", bufs=1))
            pxt = ctx1.enter_context(tc.tile_pool(name="pxt", bufs=5))
            pcs = ctx1.enter_context(tc.tile_pool(name="pcs", bufs=2))
            ptmp = ctx1.enter_context(tc.tile_pool(name="ptmp", bufs=12))

            wk_sb = p1w.tile([128, NCC, D], f32r, tag="wk")
            wkT_re = wkT.ap().rearrange("(cc p) m -> p cc m", p=128)
            for g0 in range(0, NCC, 6):
                nc.sync.dma_start(wk_sb[:, g0:g0 + 6, :], wkT_re[:, g0:g0 + 6, :])
            wv_sb = p1w.tile([128, NCC, D], f32r, tag="wv")
            wvT_re = wvT.ap().rearrange("(cc p) m -> p cc m", p=128)

            HTC = TC // 2

            def load_xt(tci):
                eng = nc.sync
                halves = []
                for hh in range(2):
                    t0 = tci * TC + hh * HTC
                    xth = pxt.tile([128, NCC, HTC], f32r, tag="xt", name="xth")
                    for g0 in range(0, NCC, 4):
                        eng.dma_start(xth[:, g0:g0 + 4, :],
                                      xT_re[:, g0:g0 + 4, t0:t0 + HTC])
                    halves.append(xth)
                return halves

            xt_next = load_xt(0)
            for g0 in range(0, NCC, 6):
                nc.scalar.dma_start(wv_sb[:, g0:g0 + 6, :], wvT_re[:, g0:g0 + 6, :])
            wq_sb = p1w.tile([128, NCC, QD], f32r, tag="wq")
            wqT_re = wqT.ap().rearrange("(cc p) m -> p cc m", p=128)
            for g0 in range(0, NCC, 3):
                nc.sync.dma_start(wq_sb[:, g0:g0 + 3, :], wqT_re[:, g0:g0 + 3, :])

            for tci in range(NTC):
                t0 = tci * TC
                xt = xt_next
                cs = pcs.tile([128, TC], f32r, tag="cs")
                nc.sync.dma_start(cs[:], cos2.ap()[:, t0:t0 + TC])
                sn = pcs.tile([128, TC], f32r, tag="sn")
                nc.sync.dma_start(sn[:], sin2m.ap()[:, t0:t0 + TC])
                ve_t = pcs.tile([128, TC], f32r, tag="vet")
                nc.sync.dma_start(ve_t[:], veT.ap()[:, t0:t0 + TC])
                if tci + 1 < NTC:
                    xt_next = load_xt(tci + 1)

                # ve gate: sigmoid(x[:, :12] @ wg); the *3 is folded into veT
                zg = psR.tile([1, TC], f32, tag="row")
                nc.tensor.matmul(zg[0:1, 0:HTC], wg_sb[:],
                                 xt[0][0:VE_GATE_CH, 0, :],
                                 start=True, stop=False)
                nc.tensor.matmul(zg[0:1, HTC:TC], wg_sb[:],
                                 xt[1][0:VE_GATE_CH, 0, :],
                                 start=False, stop=True)
                ez = prow.tile([1, TC], f32, tag="g")
                nc.scalar.activation(ez[:], zg[:], AF.Exp, scale=-1.0)
                ez1 = prow.tile([1, TC], f32, tag="g")
                nc.vector.tensor_scalar_add(ez1[:], ez[:], 1.0)
                grow = prow.tile([1, TC], f32, tag="g")
                nc.vector.reciprocal(grow[:], ez1[:])
                gbc = pbc.tile([128, TC], f32, tag="bc")
                nc.gpsimd.partition_broadcast(gbc[:], grow[:])

                streams = [("k", 0)] + [("q", m) for m in range(REP)] + [("v", 0)]
                for kind, m in streams:
                    acc = psSY.tile([128, TC], f32, tag="sy", name="acc")
                    for hh in range(2):
                        for cc in range(NCC):
                            if kind == "q":
                                lhsT = wq_sb[:, cc, m * D:(m + 1) * D]
                            elif kind == "k":
                                lhsT = wk_sb[:, cc, :]
                            else:
                                lhsT = wv_sb[:, cc, :]
                            nc.tensor.matmul(
                                acc[:, hh * HTC:(hh + 1) * HTC], lhsT,
                                xt[hh][:, cc, :],
                                start=(cc == 0 and hh == 0),
                                stop=(cc == NCC - 1 and hh == 1))

                    if kind == "v":
                        # v += gate * ve; then transpose into natural [s, D]
                        vtmp = ptmp.tile([128, TC], f32, tag="t")
                        nc.vector.tensor_mul(vtmp[:], gbc[:], ve_t[:])
                        vfull = ptmp.tile([128, TC], f32r, tag="t")
                        nc.vector.tensor_add(vfull[:], vtmp[:], acc[:])

                        def vtrans(tci=tci, vfull=vfull):
                            for j in range(TC // 128):
                                st = tci * (TC // 128) + j
                                vtr = psSY.tile([128, 128], f32r, tag="sy",
                                                name="vtr")
                                nc.tensor.transpose(
                                    vtr[:], vfull[:, j * 128:(j + 1) * 128],
                                    eye_sb[:])
                                if j % 2 == 0:
                                    nc.scalar.copy(V_sb[:, st, :], vtr[:])
                                else:
                                    nc.vector.tensor_copy(V_sb[:, st, :], vtr[:])
                        pending_pe.append(vtrans)
                        continue

                    # q/k: evacuate PSUM early, then rmsnorm stats off SBUF
                    qraw = ptmp.tile([128, TC], f32r, tag="t")
                    nc.scalar.copy(qraw[:], acc[:])
                    sqr = ptmp.tile([128, TC], f32r, tag="t")
                    nc.scalar.activation(sqr[:], qraw[:], AF.Square)

                    def final(kind=kind, m=m, qraw=qraw, sqr=sqr, t0=t0,
                              cs=cs, sn=sn):
                        ss = psR.tile([1, TC], f32, tag="row", name="ss")
                        nc.tensor.matmul(ss[:], ones_sb[:], sqr[:],
                                         start=True, stop=True)
                        lnr = prow.tile([1, TC], f32, tag="r", name="lnr")
                        nc.scalar.activation(lnr[:], ss[:], AF.Ln,
                                             scale=1.0 / D, bias=eps_row[0:1, :])
                        rr = prow.tile([1, TC], f32, tag="r", name="rr")
                        if kind == "q":
                            nc.scalar.activation(rr[:], lnr[:], AF.Exp,
                                                 scale=-0.5,
                                                 bias=lncq_row[0:1, :])
                        else:
                            nc.scalar.activation(rr[:], lnr[:], AF.Exp,
                                                 scale=-0.5, bias=0.0)
                        rbc = pbc.tile([128, TC], f32, tag="bc", name="rbc")
                        nc.gpsimd.partition_broadcast(rbc[:], rr[:])

                        qn = ptmp.tile([128, TC], f32r, tag="t", name="qn")
                        nc.vector.tensor_mul(qn[:], rbc[:], qraw[:])
                        # rope: out = qn*[cos;cos] + swap(qn)*[sin;-sin]
                        qsw = ptmp.tile([128, TC], f32r, tag="t", name="qsw")
                        nc.sync.dma_start(qsw[0:64, :], qn[64:128, :])
                        nc.sync.dma_start(qsw[64:128, :], qn[0:64, :])
                        ta = ptmp.tile([128, TC], f32r, tag="t", name="ta")
                        nc.vector.tensor_mul(ta[:], qn[:], cs[:])
                        tb = ptmp.tile([128, TC], f32r, tag="t", name="tb")
                        nc.vector.tensor_mul(tb[:], qsw[:], sn[:])
                        dst = qT_sb[m] if kind == "q" else kT_sb
                        nc.vector.tensor_add(dst[:, t0:t0 + TC], ta[:], tb[:])
                    pending_pe.append(final)

                    # keep PE dense: flush one deferred op per stream
                    if len(pending_pe) > 1:
                        flush_pe(1)
                if tci == 1:
                    nc.scalar.dma_start(
                        masks_sb[:],
                        masksI.ap().rearrange("(nd p) t -> p nd t", p=128))
            flush_pe()

        # =========== phase 2+3 per t-chunk: attention + out-proj ===========
        pw2 = ctx.enter_context(tc.tile_pool(name="pw2", bufs=1))
        wp_sb = pw2.tile([128, REP, C], f32r, tag="wp")
        nc.scalar.dma_start(wp_sb[:], wpT.ap().rearrange("(qc p) c -> p qc c",
                                                         p=128))
        yT_sb = [pw2.tile([128, T], f32r, tag=f"yT{m}", name=f"yT{m}")
                 for m in range(REP)]
        pP = ctx.enter_context(tc.tile_pool(name="pP", bufs=6))
        pout = ctx.enter_context(tc.tile_pool(name="pout", bufs=3))

        for tci in range(NTC):
            t0 = tci * TC
            if win_finite:
                st_min = max(0, (t0 - window - 127) // 128 + 1)
            else:
                st_min = 0
            st_max = (t0 + TC - 1) // 128
            sts = list(range(st_min, st_max + 1))

            for h in range(REP):
                yU = psSY.tile([128, TC], f32, tag="sy", name="yU")
                den = psR.tile([1, TC], f32, tag="row", name="den")
                q_rhs = qT_sb[h][:, t0:t0 + TC]
                pends = []    # software-pipeline den/Y two s-tiles behind
                for idx, st in enumerate(sts):
                    s0 = st * 128
                    delta = t0 - s0
                    causal_p = delta <= 0
                    window_p = win_finite and delta > window - (TC - 1)
                    nmm = int(causal_p) + int(window_p)
                    # valid column range for this s-tile (outside it every
                    # element is masked, so P is exactly 0 there and the
                    # mask-MM covers those columns of the scores bank)
                    v0 = max(0, -delta) if causal_p else 0
                    v1 = min(TC, window - delta + 128) if window_p else TC
                    if v1 - v0 < 256:      # stay on the fp32r fast path
                        v0, v1 = 0, TC
                    sc = psSY.tile([128, TC], f32, tag="sy", name="sc")
                    nc.tensor.matmul(sc[:, v0:v1], kT_sb[:, s0:s0 + 128],
                                     qT_sb[h][:, t0 + v0:t0 + v1],
                                     start=True, stop=(nmm == 0))
                    if causal_p:    # masked cols [0, 128-delta)
                        c0, c1 = 0, min(TC, 128 - delta)
                        off = delta + (TC - 128)
                        nmm -= 1
                        nc.tensor.matmul(sc[:, c0:c1], eye_sb[:],
                                         masks_sb[:, 0, off + c0:off + c1],
                                         start=False, stop=(nmm == 0))
                    if window_p:    # masked cols suffix
                        c0 = min(TC - 128, (window - delta + 1) // 128 * 128)
                        c1 = TC
                        off = delta - wmin
                        nmm -= 1
                        nc.tensor.matmul(sc[:, c0:c1], eye_sb[:],
                                         masks_sb[:, 1, off + c0:off + c1],
                                         start=False, stop=(nmm == 0))
                    if len(pends) >= 2:
                        pends.pop(0)()
                    P = pP.tile([128, TC], f32r, tag="P", name="P")
                    nc.scalar.activation(P[:], sc[:], AF.Exp)

                    def mk(idx=idx, st=st, P=P, v0=v0, v1=v1):
                        first, last = idx == 0, idx == len(sts) - 1
                        def go():
                            nc.tensor.matmul(den[0:1, v0:v1], ones_sb[:],
                                             P[:, v0:v1],
                                             start=first, stop=last)
                            nc.tensor.matmul(yU[:, v0:v1], V_sb[:, st, :],
                                             P[:, v0:v1],
                                             start=first, stop=last)
                        return go
                    pends.append(mk())
                while pends:
                    pends.pop(0)()
                dinv = prow.tile([1, TC], f32, tag="r", name="dinv")
                nc.vector.reciprocal(dinv[:], den[:])
                dbc = pbc.tile([128, TC], f32, tag="bc", name="dbc")
                nc.gpsimd.partition_broadcast(dbc[:], dinv[:])
                nc.vector.tensor_mul(yT_sb[h][:, t0:t0 + TC], dbc[:], yU[:])

            # ---- out-proj for this t-chunk ----
            for cc in range(NCC):
                if tci == NTC - 1 and cc % 2 == 1:
                    o = psSY.tile([128, TC], f32, tag="sy", name="o")
                else:
                    o = psAO.tile([128, TC], f32, tag="ao", name="o")
                for m in range(REP):
                    nc.tensor.matmul(o[:], wp_sb[:, m, cc * 128:(cc + 1) * 128],
                                     yT_sb[m][:, t0:t0 + TC],
                                     start=(m == 0), stop=(m == REP - 1))
                ot = pout.tile([128, TC], f32, tag="ot", name="ot")
                if cc % 2 == 0:
                    nc.vector.tensor_copy(ot[:], o[:])
                else:
                    nc.scalar.copy(ot[:], o[:])
                nc.sync.dma_start(outT.ap()[cc * 128:(cc + 1) * 128,
                                            t0:t0 + TC], ot[:])

    nc.compile()
    nc._mask_cfg = {"wide": WIDE, "cmin": -(TC - 128), "wmin": wmin}
    return nc


def _prep_inputs(nc, window, x, ve, cos, sin, Wq, Wk, Wv, Wproj, Wg):
    """Build the 8 per-core input maps (host-side sharding + transposes)."""
    win_finite = 0 <= window < T
    cosT = np.ascontiguousarray(cos.reshape(T, D // 2).T)
    sinT = np.ascontiguousarray(sin.reshape(T, D // 2).T)
    cos2 = np.concatenate([cosT, cosT], axis=0)
    sin2m = np.concatenate([sinT, -sinT], axis=0)
    eye = np.eye(128, dtype=np.float32)
    ones = np.ones((128, 1), dtype=np.float32)

    ds = np.arange(128)[:, None]
    wcfg = nc._mask_cfg
    j = np.arange(wcfg["wide"])[None, :]
    mc = np.where(j + wcfg["cmin"] - ds >= 0, 0.0, _MASKVAL).astype(np.float32)
    rows = [mc]
    if win_finite:
        mw = np.where(j + wcfg["wmin"] - ds <= window, 0.0,
                      _MASKVAL).astype(np.float32)
        rows.append(mw)
    masks = np.concatenate(rows, axis=0)

    xTb = [np.ascontiguousarray(x[b].T) for b in range(B)]

    in_maps = []
    for core in range(N_CORES):
        b, g = divmod(core, KV)
        sl_q = slice(g * QD, (g + 1) * QD)
        sl_d = slice(g * D, (g + 1) * D)
        in_maps.append({
            "xT": xTb[b],
            "wqT": np.ascontiguousarray(Wq[sl_q].T),
            "wkT": np.ascontiguousarray(Wk[sl_d].T),
            "wvT": np.ascontiguousarray(Wv[sl_d].T),
            "wpT": np.ascontiguousarray(Wproj[:, sl_q].T),
            "wg": np.ascontiguousarray(Wg[g].reshape(VE_GATE_CH, 1)),
            "veT": np.ascontiguousarray(3.0 * ve[b, :, sl_d].T),
            "cos2": cos2, "sin2m": sin2m, "eye": eye, "onesI": ones,
            "masksI": masks,
        })
    return in_maps


def kernel(x, ve, cos, sin, Wq, Wk, Wv, Wproj, Wg, window, _trace=False):
    window = int(window)
    if window not in _CACHE:
        _CACHE[window] = _build(window)
    nc = _CACHE[window]

    in_maps = _prep_inputs(nc, window,
                           np.asarray(x, np.float32), np.asarray(ve, np.float32),
                           np.asarray(cos, np.float32), np.asarray(sin, np.float32),
                           np.asarray(Wq, np.float32), np.asarray(Wk, np.float32),
                           np.asarray(Wv, np.float32), np.asarray(Wproj, np.float32),
                           np.asarray(Wg, np.float32))

    res = run_bass_kernel_spmd(nc, in_maps, core_ids=list(range(N_CORES)),
                               trace=_trace)

    out = np.empty((B, T, C), dtype=np.float32)
    for b in range(B):
        acc = res.results[b * KV]["outT"].copy()
        for g in range(1, KV):
            acc += res.results[b * KV + g]["outT"]
        out[b] = acc.T
    if _trace:
        kernel._last_trace = res
    return out


# revision 72
# speedup vs baseline: 1.0443x; 1.0106x over previous
"""Trainium2 Bass kernel for GQA causal sliding-window self-attention.

Sharding: 8 cores = 2 (batch) x 4 (KV-head groups). Each core handles one
batch element and one KV head with its 3 GQA query heads. The output
projection is computed per-group against the matching Wproj column slice;
the 4 partial outputs per batch are summed on the host.

Everything on-chip runs in feature-major ("transposed") layout so that all
matmul contractions have their contraction dim on SBUF partitions and all
DRAM traffic is contiguous. fp32r matmuls (full-rate) with fp32 PSUM
accumulation. Sliding-window/causal masking is applied by accumulating a
host-precomputed -1e9 additive mask tile into the scores PSUM via an
identity matmul (exp then underflows to exactly 0).
"""

import os
import sys
import numpy as np

sys.path.insert(0, "/opt/trn_rl_repo")

from contextlib import ExitStack

from concourse import mybir, bacc, tile
from concourse.bass_utils import run_bass_kernel_spmd

f32 = mybir.dt.float32
f32r = mybir.dt.float32r
AF = mybir.ActivationFunctionType

B, T, C = 2, 2048, 1536
H, KV, D = 12, 4, 128
REP = H // KV          # 3 query heads per kv head
QD = REP * D           # 384
VE_GATE_CH = 12
N_CORES = 8
TC = 512               # t-chunk width (matmul moving free dim)
NTC = T // TC          # 4
NCC = C // 128         # 12 contraction chunks
NST = T // 128         # 16 s-tiles

_EPS = float(np.finfo(np.float32).eps)
# all scale constants folded into the q-side rsqrt:
#   rq = (1.2*1.2/sqrt(D)) * rsqrt(mean(q^2)+eps),  rk = rsqrt(mean(k^2)+eps)
_LNCQ = float(np.log(1.2 * 1.2 / np.sqrt(D)))
_MASKVAL = -1.0e9

_CACHE = {}


def _setup_act_tables():
    """Reorder activation-table sets so ln+exp share one set (avoids ~33
    table reloads).  Patches both the bacc-side set picker and the walrus
    --act-root-json (they must agree on set indices)."""
    try:
        import json
        import tempfile
        import concourse.hw_specs as hw_specs
        import concourse.bacc as bacc_mod
        from neuronxcc.driver.Job import Job
        from neuronxcc.driver.jobs.support.FindActInfo import findActInfoFile

        src = findActInfoFile(Job.getPackageDir(), "gen3")
        if not src or not os.path.exists(src):
            return
        src_dir = os.path.dirname(src)
        dst = os.path.join(tempfile.gettempdir(), "bass_act_pwp_lnexp")
        os.makedirs(dst, exist_ok=True)
        for f in os.listdir(src_dir):
            tgt = os.path.join(dst, f)
            if not os.path.exists(tgt):
                try:
                    os.symlink(os.path.join(src_dir, f), tgt)
                except OSError:
                    pass
        d = json.load(open(src))
        sets = d["act_func_sets"]
        idx = [i for i, s in enumerate(sets)
               if s["name"] == "natural_log_exp_and_others"]
        if not idx:
            return
        sets.insert(0, sets.pop(idx[0]))
        jp = os.path.join(dst, "act_info.json")
        if os.path.lexists(jp):
            os.remove(jp)
        json.dump(d, open(jp, "w"))
        os.environ["BASS_ACT_ROOT_JSON_PATH"] = jp

        orig = hw_specs.get_activation_tables

        def reordered(arch):
            t = orig(arch)
            key = "natural_log_exp_and_others"
            if key in t:
                out = {key: t[key]}
                out.update((k, v) for k, v in t.items() if k != key)
                return out
            return t

        hw_specs.get_activation_tables = reordered
        bacc_mod.get_activation_tables = reordered
    except Exception:
        pass


_setup_act_tables()


def _partial_deltas(window, win_finite):
    """Tile-offset classes (delta = t0 - s0) that need an additive mask."""
    deltas = set()
    for dlt in range(-(TC - 128), 0 + 1, 128):        # causal partials
        deltas.add(dlt)
    if win_finite:
        dlt = window - (window % 128)                  # window partials
        while dlt + (TC - 1) > window:
            if dlt >= -(TC - 128):
                deltas.add(dlt)
            dlt -= 128
    return sorted(deltas)


def _build(window: int):
    win_finite = 0 <= window < T
    deltas = _partial_deltas(window, win_finite)
    wdeltas = [d for d in deltas if win_finite and d > window - (TC - 1)]
    wmin = min(wdeltas) if wdeltas else 0
    WIDE = TC + (TC - 128)                  # covers 4 deltas of 128
    NM = 2 if wdeltas else 1

    nc = bacc.Bacc("TRN2", target_bir_lowering=False, debug=False,
                   num_devices=N_CORES)

    xT = nc.dram_tensor("xT", [C, T], f32r, kind="ExternalInput")
    wqT = nc.dram_tensor("wqT", [C, QD], f32r, kind="ExternalInput")
    wkT = nc.dram_tensor("wkT", [C, D], f32r, kind="ExternalInput")
    wvT = nc.dram_tensor("wvT", [C, D], f32r, kind="ExternalInput")
    wpT = nc.dram_tensor("wpT", [QD, C], f32r, kind="ExternalInput")
    wg = nc.dram_tensor("wg", [VE_GATE_CH, 1], f32r, kind="ExternalInput")
    veT = nc.dram_tensor("veT", [D, T], f32r, kind="ExternalInput")
    cos2 = nc.dram_tensor("cos2", [128, T], f32r, kind="ExternalInput")
    sin2m = nc.dram_tensor("sin2m", [128, T], f32r, kind="ExternalInput")
    eye = nc.dram_tensor("eye", [128, 128], f32r, kind="ExternalInput")
    onesI = nc.dram_tensor("onesI", [128, 1], f32r, kind="ExternalInput")
    masksI = nc.dram_tensor("masksI", [NM * 128, WIDE], f32r, kind="ExternalInput")
    outT = nc.dram_tensor("outT", [C, T], f32, kind="ExternalOutput")

    with tile.TileContext(nc) as tc, ExitStack() as ctx:
        # ---- persistent SBUF pools ----
        pw = ctx.enter_context(tc.tile_pool(name="pw", bufs=1))
        pbig = ctx.enter_context(tc.tile_pool(name="pbig", bufs=1))
        prow = ctx.enter_context(tc.tile_pool(name="prow", bufs=6))
        pbc = ctx.enter_context(tc.tile_pool(name="pbc", bufs=4))

        # ---- PSUM pools (8 banks total, elastic shared tags) ----
        psAO = ctx.enter_context(tc.tile_pool(name="psAO", bufs=1, space="PSUM"))
        psR = ctx.enter_context(tc.tile_pool(name="psR", bufs=2, space="PSUM"))
        psSY = ctx.enter_context(tc.tile_pool(name="psSY", bufs=5, space="PSUM"))

        # small constants (needed from phase 1)
        wg_sb = pw.tile([VE_GATE_CH, 1], f32r, tag="wg")
        nc.sync.dma_start(wg_sb[:], wg.ap()[:])
        ones_sb = pw.tile([128, 1], f32r, tag="ones")
        nc.sync.dma_start(ones_sb[:], onesI.ap()[:])
        eye_sb = pw.tile([128, 128], f32r, tag="eye")
        nc.sync.dma_start(eye_sb[:], eye.ap()[:])
        masks_sb = pw.tile([128, NM, WIDE], f32r, tag="masks")
        eps_row = pw.tile([128, 1], f32, tag="epsr")
        nc.vector.memset(eps_row[:], _EPS)
        lncq_row = pw.tile([128, 1], f32, tag="lncq")
        nc.vector.memset(lncq_row[:], _LNCQ)

        # big persistent activations
        qT_sb = [pbig.tile([128, T], f32r, tag=f"qT{m}", name=f"qT{m}")
                 for m in range(REP)]
        kT_sb = pbig.tile([128, T], f32r, tag="kT")
        V_sb = pbig.tile([128, NST, D], f32r, tag="V")

        xT_re = xT.ap().rearrange("(cc p) t -> p cc t", p=128)

        # =========== phase 1: projections + gate + rope + rmsnorm ===========
        pending_pe = []   # deferred PE ops (sumsq matmuls, v transposes)

        def flush_pe(n=None):
            k = len(pending_pe) if n is None else min(n, len(pending_pe))
            for _ in range(k):
                pending_pe.pop(0)()

        with ExitStack() as ctx1:
            p1w = ctx1.enter_context(tc.tile_pool(name="p1w", bufs=1))
            pxt = ctx1.enter_context(tc.tile_pool(name="pxt", bufs=5))
            pcs = ctx1.enter_context(tc.tile_pool(name="pcs", bufs=2))
            ptmp = ctx1.enter_context(tc.tile_pool(name="ptmp", bufs=12))

            wk_sb = p1w.tile([128, NCC, D], f32r, tag="wk")
            wkT_re = wkT.ap().rearrange("(cc p) m -> p cc m", p=128)
            for g0 in range(0, NCC, 6):
                nc.sync.dma_start(wk_sb[:, g0:g0 + 6, :], wkT_re[:, g0:g0 + 6, :])
            wv_sb = p1w.tile([128, NCC, D], f32r, tag="wv")
            wvT_re = wvT.ap().rearrange("(cc p) m -> p cc m", p=128)

            HTC = TC // 2

            def load_xt(tci):
                eng = nc.sync
                halves = []
                for hh in range(2):
                    t0 = tci * TC + hh * HTC
                    xth = pxt.tile([128, NCC, HTC], f32r, tag="xt", name="xth")
                    for g0 in range(0, NCC, 4):
                        eng.dma_start(xth[:, g0:g0 + 4, :],
                                      xT_re[:, g0:g0 + 4, t0:t0 + HTC])
                    halves.append(xth)
                return halves

            xt_next = load_xt(0)
            for g0 in range(0, NCC, 6):
                nc.scalar.dma_start(wv_sb[:, g0:g0 + 6, :], wvT_re[:, g0:g0 + 6, :])
            wq_sb = p1w.tile([128, NCC, QD], f32r, tag="wq")
            wqT_re = wqT.ap().rearrange("(cc p) m -> p cc m", p=128)
            for g0 in range(0, NCC, 3):
                nc.sync.dma_start(wq_sb[:, g0:g0 + 3, :], wqT_re[:, g0:g0 + 3, :])

            for tci in range(NTC):
                t0 = tci * TC
                xt = xt_next
                cs = pcs.tile([128, TC], f32r, tag="cs")
                nc.sync.dma_start(cs[:], cos2.ap()[:, t0:t0 + TC])
                sn = pcs.tile([128, TC], f32r, tag="sn")
                nc.sync.dma_start(sn[:], sin2m.ap()[:, t0:t0 + TC])
                ve_t = pcs.tile([128, TC], f32r, tag="vet")
                nc.sync.dma_start(ve_t[:], veT.ap()[:, t0:t0 + TC])
                if tci + 1 < NTC:
                    xt_next = load_xt(tci + 1)

                # ve gate: sigmoid(x[:, :12] @ wg); the *3 is folded into veT
                zg = psR.tile([1, TC], f32, tag="row")
                nc.tensor.matmul(zg[0:1, 0:HTC], wg_sb[:],
                                 xt[0][0:VE_GATE_CH, 0, :],
                                 start=True, stop=False)
                nc.tensor.matmul(zg[0:1, HTC:TC], wg_sb[:],
                                 xt[1][0:VE_GATE_CH, 0, :],
                                 start=False, stop=True)
                ez = prow.tile([1, TC], f32, tag="g")
                nc.scalar.activation(ez[:], zg[:], AF.Exp, scale=-1.0)
                ez1 = prow.tile([1, TC], f32, tag="g")
                nc.vector.tensor_scalar_add(ez1[:], ez[:], 1.0)
                grow = prow.tile([1, TC], f32, tag="g")
                nc.vector.reciprocal(grow[:], ez1[:])
                gbc = pbc.tile([128, TC], f32, tag="bc")
                nc.gpsimd.partition_broadcast(gbc[:], grow[:])

                streams = [("k", 0)] + [("q", m) for m in range(REP)] + [("v", 0)]
                for kind, m in streams:
                    acc = psSY.tile([128, TC], f32, tag="sy", name="acc")
                    for hh in range(2):
                        for cc in range(NCC):
                            if kind == "q":
                                lhsT = wq_sb[:, cc, m * D:(m + 1) * D]
                            elif kind == "k":
                                lhsT = wk_sb[:, cc, :]
                            else:
                                lhsT = wv_sb[:, cc, :]
                            nc.tensor.matmul(
                                acc[:, hh * HTC:(hh + 1) * HTC], lhsT,
                                xt[hh][:, cc, :],
                                start=(cc == 0 and hh == 0),
                                stop=(cc == NCC - 1 and hh == 1))

                    if kind == "v":
                        # v += gate * ve; then transpose into natural [s, D]
                        vtmp = ptmp.tile([128, TC], f32, tag="t")
                        nc.vector.tensor_mul(vtmp[:], gbc[:], ve_t[:])
                        vfull = ptmp.tile([128, TC], f32r, tag="t")
                        nc.vector.tensor_add(vfull[:], vtmp[:], acc[:])

                        def vtrans(tci=tci, vfull=vfull):
                            for j in range(TC // 128):
                                st = tci * (TC // 128) + j
                                vtr = psSY.tile([128, 128], f32r, tag="sy",
                                                name="vtr")
                                nc.tensor.transpose(
                                    vtr[:], vfull[:, j * 128:(j + 1) * 128],
                                    eye_sb[:])
                                if j % 2 == 0:
                                    nc.scalar.copy(V_sb[:, st, :], vtr[:])
                                else:
                                    nc.vector.tensor_copy(V_sb[:, st, :], vtr[:])
                        pending_pe.append(vtrans)
                        continue

                    # q/k: evacuate PSUM early, then rmsnorm stats off SBUF
                    qraw = ptmp.tile([128, TC], f32r, tag="t")
                    nc.scalar.copy(qraw[:], acc[:])
                    sqr = ptmp.tile([128, TC], f32r, tag="t")
                    nc.scalar.activation(sqr[:], qraw[:], AF.Square)

                    def final(kind=kind, m=m, qraw=qraw, sqr=sqr, t0=t0,
                              cs=cs, sn=sn):
                        ss = psR.tile([1, TC], f32, tag="row", name="ss")
                        nc.tensor.matmul(ss[:], ones_sb[:], sqr[:],
                                         start=True, stop=True)
                        lnr = prow.tile([1, TC], f32, tag="r", name="lnr")
                        nc.scalar.activation(lnr[:], ss[:], AF.Ln,
                                             scale=1.0 / D, bias=eps_row[0:1, :])
                        rr = prow.tile([1, TC], f32, tag="r", name="rr")
                        if kind == "q":
                            nc.scalar.activation(rr[:], lnr[:], AF.Exp,
                                                 scale=-0.5,
                                                 bias=lncq_row[0:1, :])
                        else:
                            nc.scalar.activation(rr[:], lnr[:], AF.Exp,
                                                 scale=-0.5, bias=0.0)
                        rbc = pbc.tile([128, TC], f32, tag="bc", name="rbc")
                        nc.gpsimd.partition_broadcast(rbc[:], rr[:])

                        qn = ptmp.tile([128, TC], f32r, tag="t", name="qn")
                        nc.vector.tensor_mul(qn[:], rbc[:], qraw[:])
                        # rope: out = qn*[cos;cos] + swap(qn)*[sin;-sin]
                        qsw = ptmp.tile([128, TC], f32r, tag="t", name="qsw")
                        nc.sync.dma_start(qsw[0:64, :], qn[64:128, :])
                        nc.sync.dma_start(qsw[64:128, :], qn[0:64, :])
                        ta = ptmp.tile([128, TC], f32r, tag="t", name="ta")
                        nc.vector.tensor_mul(ta[:], qn[:], cs[:])
                        tb = ptmp.tile([128, TC], f32r, tag="t", name="tb")
                        nc.vector.tensor_mul(tb[:], qsw[:], sn[:])
                        dst = qT_sb[m] if kind == "q" else kT_sb
                        nc.vector.tensor_add(dst[:, t0:t0 + TC], ta[:], tb[:])
                    pending_pe.append(final)

                    # keep PE dense: flush one deferred op per stream
                    if len(pending_pe) > 1:
                        flush_pe(1)
                if tci == 1:
                    nc.scalar.dma_start(
                        masks_sb[:],
                        masksI.ap().rearrange("(nd p) t -> p nd t", p=128))
            flush_pe()

        # =========== phase 2+3 per t-chunk: attention + out-proj ===========
        pw2 = ctx.enter_context(tc.tile_pool(name="pw2", bufs=1))
        wp_sb = pw2.tile([128, REP, C], f32r, tag="wp")
        nc.scalar.dma_start(wp_sb[:], wpT.ap().rearrange("(qc p) c -> p qc c",
                                                         p=128))
        yT_sb = [pw2.tile([128, T], f32r, tag=f"yT{m}", name=f"yT{m}")
                 for m in range(REP)]
        pP = ctx.enter_context(tc.tile_pool(name="pP", bufs=6))
        pout = ctx.enter_context(tc.tile_pool(name="pout", bufs=3))

        for tci in range(NTC):
            t0 = tci * TC
            if win_finite:
                st_min = max(0, (t0 - window - 127) // 128 + 1)
            else:
                st_min = 0
            st_max = (t0 + TC - 1) // 128
            sts = list(range(st_min, st_max + 1))

            for h in range(REP):
                yU = psSY.tile([128, TC], f32, tag="sy", name="yU")
                den = psR.tile([1, TC], f32, tag="row", name="den")
                q_rhs = qT_sb[h][:, t0:t0 + TC]
                pends = []    # software-pipeline den/Y two s-tiles behind
                for idx, st in enumerate(sts):
                    s0 = st * 128
                    delta = t0 - s0
                    causal_p = delta <= 0
                    window_p = win_finite and delta > window - (TC - 1)
                    nmm = int(causal_p) + int(window_p)
                    # valid column range for this s-tile (outside it every
                    # element is masked, so P is exactly 0 there and the
                    # mask-MM covers those columns of the scores bank)
                    v0 = max(0, -delta) if causal_p else 0
                    v1 = min(TC, window - delta + 128) if window_p else TC
                    if v1 - v0 < 256:      # stay on the fp32r fast path
                        v0, v1 = 0, TC
                    sc = psSY.tile([128, TC], f32, tag="sy", name="sc")
                    nc.tensor.matmul(sc[:, v0:v1], kT_sb[:, s0:s0 + 128],
                                     qT_sb[h][:, t0 + v0:t0 + v1],
                                     start=True, stop=(nmm == 0))
                    if causal_p:    # masked cols [0, 128-delta)
                        c0, c1 = 0, max(256, min(TC, 128 - delta))
                        off = delta + (TC - 128)
                        nmm -= 1
                        nc.tensor.matmul(sc[:, c0:c1], eye_sb[:],
                                         masks_sb[:, 0, off + c0:off + c1],
                                         start=False, stop=(nmm == 0))
                    if window_p:    # masked cols suffix
                        c0 = min(TC - 256,
                                 (window - delta + 1) // 128 * 128)
                        c1 = TC
                        off = delta - wmin
                        nmm -= 1
                        nc.tensor.matmul(sc[:, c0:c1], eye_sb[:],
                                         masks_sb[:, 1, off + c0:off + c1],
                                         start=False, stop=(nmm == 0))
                    if len(pends) >= 2:
                        pends.pop(0)()
                    P = pP.tile([128, TC], f32r, tag="P", name="P")
                    nc.scalar.activation(P[:, v0:v1], sc[:, v0:v1], AF.Exp)

                    def mk(idx=idx, st=st, P=P, v0=v0, v1=v1):
                        first, last = idx == 0, idx == len(sts) - 1
                        def go():
                            nc.tensor.matmul(den[0:1, v0:v1], ones_sb[:],
                                             P[:, v0:v1],
                                             start=first, stop=last)
                            nc.tensor.matmul(yU[:, v0:v1], V_sb[:, st, :],
                                             P[:, v0:v1],
                                             start=first, stop=last)
                        return go
                    pends.append(mk())
                while pends:
                    pends.pop(0)()
                dinv = prow.tile([1, TC], f32, tag="r", name="dinv")
                nc.vector.reciprocal(dinv[:], den[:])
                dbc = pbc.tile([128, TC], f32, tag="bc", name="dbc")
                nc.gpsimd.partition_broadcast(dbc[:], dinv[:])
                nc.vector.tensor_mul(yT_sb[h][:, t0:t0 + TC], dbc[:], yU[:])

            # ---- out-proj for this t-chunk ----
            for cc in range(NCC):
                if tci == NTC - 1 and cc % 2 == 1:
                    o = psSY.tile([128, TC], f32, tag="sy", name="o")
                else:
                    o = psAO.tile([128, TC], f32, tag="ao", name="o")
                for m in range(REP):
                    nc.tensor.matmul(o[:], wp_sb[:, m, cc * 128:(cc + 1) * 128],
                                     yT_sb[m][:, t0:t0 + TC],
                                     start=(m == 0), stop=(m == REP - 1))
                ot = pout.tile([128, TC], f32, tag="ot", name="ot")
                if cc % 2 == 0:
                    nc.vector.tensor_copy(ot[:], o[:])
                else:
                    nc.scalar.copy(ot[:], o[:])
                nc.sync.dma_start(outT.ap()[cc * 128:(cc + 1) * 128,
                                            t0:t0 + TC], ot[:])

    nc.compile()
    nc._mask_cfg = {"wide": WIDE, "cmin": -(TC - 128), "wmin": wmin}
    return nc


def _prep_inputs(nc, window, x, ve, cos, sin, Wq, Wk, Wv, Wproj, Wg):
    """Build the 8 per-core input maps (host-side sharding + transposes)."""
    win_finite = 0 <= window < T
    cosT = np.ascontiguousarray(cos.reshape(T, D // 2).T)
    sinT = np.ascontiguousarray(sin.reshape(T, D // 2).T)
    cos2 = np.concatenate([cosT, cosT], axis=0)
    sin2m = np.concatenate([sinT, -sinT], axis=0)
    eye = np.eye(128, dtype=np.float32)
    ones = np.ones((128, 1), dtype=np.float32)

    ds = np.arange(128)[:, None]
    wcfg = nc._mask_cfg
    j = np.arange(wcfg["wide"])[None, :]
    mc = np.where(j + wcfg["cmin"] - ds >= 0, 0.0, _MASKVAL).astype(np.float32)
    rows = [mc]
    if win_finite:
        mw = np.where(j + wcfg["wmin"] - ds <= window, 0.0,
                      _MASKVAL).astype(np.float32)
        rows.append(mw)
    masks = np.concatenate(rows, axis=0)

    xTb = [np.ascontiguousarray(x[b].T) for b in range(B)]

    in_maps = []
    for core in range(N_CORES):
        b, g = divmod(core, KV)
        sl_q = slice(g * QD, (g + 1) * QD)
        sl_d = slice(g * D, (g + 1) * D)
        in_maps.append({
            "xT": xTb[b],
            "wqT": np.ascontiguousarray(Wq[sl_q].T),
            "wkT": np.ascontiguousarray(Wk[sl_d].T),
            "wvT": np.ascontiguousarray(Wv[sl_d].T),
            "wpT": np.ascontiguousarray(Wproj[:, sl_q].T),
            "wg": np.ascontiguousarray(Wg[g].reshape(VE_GATE_CH, 1)),
            "veT": np.ascontiguousarray(3.0 * ve[b, :, sl_d].T),
            "cos2": cos2, "sin2m": sin2m, "eye": eye, "onesI": ones,
            "masksI": masks,
        })
    return in_maps


def kernel(x, ve, cos, sin, Wq, Wk, Wv, Wproj, Wg, window, _trace=False):
    window = int(window)
    if window not in _CACHE:
        _CACHE[window] = _build(window)
    nc = _CACHE[window]

    in_maps = _prep_inputs(nc, window,
                           np.asarray(x, np.float32), np.asarray(ve, np.float32),
                           np.asarray(cos, np.float32), np.asarray(sin, np.float32),
                           np.asarray(Wq, np.float32), np.asarray(Wk, np.float32),
                           np.asarray(Wv, np.float32), np.asarray(Wproj, np.float32),
                           np.asarray(Wg, np.float32))

    res = run_bass_kernel_spmd(nc, in_maps, core_ids=list(range(N_CORES)),
                               trace=_trace)

    out = np.empty((B, T, C), dtype=np.float32)
    for b in range(B):
        acc = res.results[b * KV]["outT"].copy()
        for g in range(1, KV):
            acc += res.results[b * KV + g]["outT"]
        out[b] = acc.T
    if _trace:
        kernel._last_trace = res
    return out
